# revision 17
# baseline (speedup 1.0000x reference)
"""Distributed GINE GNN kernel for 8 Trainium2 NeuronCores.

Sharding: nodes partitioned contiguously across cores (12500/core, padded to
12544 = 98 windows of 128); edges assigned to the core owning their dst;
src features read from a replicated bf16 copy of h, AllGather'd per layer.

Edges are bucketed host-side by dst window (128 nodes). Per 128-edge chunk:
  e   = [attr|1] @ [We;be]          (PE, K=17)
  e  += h_full[src]                 (gather via indirect DMA + PE identity-matmul
                                     accumulate into the e PSUM bank)
  m   = relu(e)                     (ACT: fused relu on PSUM->SBUF eviction)
  agg[dst] += m                     (PE: one-hot matmul, accumulated in PSUM
                                     over all chunks of the window)
One-hot tiles are built on DVE via tensor_scalar is_equal against an iota row.
Node MLP runs D-major (BN folded into W1); PE transposes convert between
node-major and D-major. Pooling = one-hot matmuls by graph id (fp32 PSUM),
assembled into a global buffer by indirect row scatter, AllReduce'd; the
classifier runs redundantly on every core.
"""

import numpy as np
import ml_dtypes

import concourse.bass as bass
import concourse.bacc as bacc
import concourse.mybir as mybir
import concourse.tile as tile
from concourse import bass_utils

# ---------------- problem constants ----------------
N = 100000
E = 1600000
D = 128
ED = 16
L = 3
G = 1024
C = 10
BN_EPS = 1e-5

CORES = 8
NPC = N // CORES          # 12500
NLOC = 12544              # 98 * 128
NW = NLOC // 128          # 98 dst windows
HROWS = CORES * NLOC      # 100352 rows of replicated h

GLW = 256                 # local graph-id window span (2 psum tiles of 128)

BF16 = mybir.dt.bfloat16
F32 = mybir.dt.float32
I16 = mybir.dt.int16
I32 = mybir.dt.int32

bf16 = ml_dtypes.bfloat16


# ---------------- host-side prep ----------------

def prepare(x, edge_attr, We, be, W1, b1, gamma, beta, W2, b2,
            Wc1, bc1, Wc2, bc2, edge_index, batch):
    x = np.asarray(x, np.float32)
    edge_attr = np.asarray(edge_attr, np.float32)
    edge_index = np.asarray(edge_index, np.int64)
    batch = np.asarray(batch, np.int64)

    rstd = 1.0 / np.sqrt(1.0 + BN_EPS)
    s = rstd * np.asarray(gamma, np.float32)
    W1f = np.asarray(W1, np.float32) * s[:, None, :]
    b1f = np.asarray(b1, np.float32) * s + np.asarray(beta, np.float32)

    src, dst = edge_index[0], edge_index[1]
    sc = src // NPC
    src_row = (sc * NLOC + (src - sc * NPC)).astype(np.int32)
    core_of_edge = dst // NPC
    dst_local = dst - core_of_edge * NPC

    # bucket edges by (core, dst window); SPW = max bucket size rounded to 128
    key = core_of_edge * NW + dst_local // 128
    order = np.argsort(key, kind="stable")
    key_s = key[order]
    bounds = np.searchsorted(key_s, np.arange(CORES * NW + 1))
    SPW = int((np.diff(bounds).max() + 127) // 128 * 128)
    EC = NW * SPW
    NCH = EC // 128                      # chunks per core

    g_rows = np.zeros((CORES, EC), np.int32)
    dstrel = np.full((CORES, EC), -1, np.int32)
    attr_slots = np.zeros((CORES, EC, ED), np.float32)
    for c in range(CORES):
        for w in range(NW):
            lo, hi = bounds[c * NW + w], bounds[c * NW + w + 1]
            eids = order[lo:hi]
            base = w * SPW
            n = hi - lo
            g_rows[c, base:base + n] = src_row[eids]
            dstrel[c, base:base + n] = dst_local[eids] % 128
            attr_slots[c, base:base + n] = edge_attr[eids]

    # slot s holds edge at (partition s%128, chunk s//128)
    def wrap_cols(a):  # [CORES, EC] -> [CORES, 128, NCH]
        return np.transpose(a.reshape(CORES, NCH, 128), (0, 2, 1))

    g_off = np.ascontiguousarray(wrap_cols(g_rows))
    dstrel_w = np.ascontiguousarray(wrap_cols(dstrel)).astype(np.float32)

    attrT = np.ones((CORES, ED + 1, EC), np.float32)
    attrT[:, :ED, :] = np.transpose(attr_slots, (0, 2, 1))
    attrT = attrT.astype(bf16)

    # x: replicated storage rows + per-core node-major wrapped local panel
    x_full = np.zeros((HROWS, D), np.float32)
    x_locN = np.zeros((CORES, 128, NLOC), np.float32)
    for c in range(CORES):
        xc = x[c * NPC:(c + 1) * NPC]
        x_full[c * NLOC: c * NLOC + NPC] = xc
        xp = np.zeros((NLOC, D), np.float32)
        xp[:NPC] = xc
        # node i at partition i%128, cols (i//128)*128 : +128
        x_locN[c] = xp.reshape(NW, 128, D).transpose(1, 0, 2).reshape(128, NLOC)
    x_full = x_full.astype(bf16)
    x_locN = x_locN.astype(bf16)

    # pooling: glocal[p, w] = batch[local node w*128+p] - gbase (pad -> -1)
    glocal = np.full((CORES, 128, NW), -1, np.float32)
    pool_rows = np.zeros((CORES, 128, 2), np.int32)
    for c in range(CORES):
        bb = batch[c * NPC:(c + 1) * NPC]
        gb = int(bb[0])
        span = int(bb[-1] - bb[0])
        assert span < GLW, f"graph span {span} exceeds {GLW}"
        gl = np.full(NLOC, -1, np.int64)
        gl[:NPC] = bb - gb
        glocal[c] = gl.reshape(NW, 128).T.astype(np.float32)
        pool_rows[c, :, 0] = gb + np.arange(128)
        pool_rows[c, :, 1] = gb + 128 + np.arange(128)
    pool_rows = np.clip(pool_rows, 0, G + GLW - 1).astype(np.int32)

    weights = dict(
        WeT=np.ascontiguousarray(np.asarray(We, np.float32)).astype(bf16),
        beb=np.asarray(be, np.float32).astype(bf16),
        W1f=W1f.astype(bf16), W2=np.asarray(W2, np.float32).astype(bf16),
        b1f=b1f.astype(np.float32), b2=np.asarray(b2, np.float32),
        Wc1=np.asarray(Wc1, np.float32).astype(bf16),
        Wc2=np.asarray(Wc2, np.float32).astype(bf16),
        bc1=np.asarray(bc1, np.float32), bc2=np.asarray(bc2, np.float32),
    )
    aux = dict(
        iota=np.tile(np.arange(128, dtype=np.float32), (128, 1)),
        iota2=np.tile(np.arange(GLW, dtype=np.float32), (128, 1)),
        ident=np.eye(128, dtype=np.float32).astype(bf16),
    )
    return dict(SPW=SPW, EC=EC, g_off=g_off, dstrel=dstrel_w, attrT=attrT,
                x_full=x_full, x_locN=x_locN, glocal=glocal,
                pool_rows=pool_rows, weights=weights, aux=aux)


# ---------------- device program ----------------

def build_program(SPW):
    nc = bacc.Bacc("TRN2", target_bir_lowering=False, debug=False,
                   num_devices=CORES, num_swdge_queues=4)
    EC = NW * SPW
    NCH = EC // 128
    CPW = SPW // 128                    # chunks per window
    SUB = 4                             # chunks per psum e-bank

    t_xfull = nc.dram_tensor("x_full", [HROWS, D], BF16, kind="ExternalInput")
    t_xlocN = nc.dram_tensor("x_locN", [128, NLOC], BF16, kind="ExternalInput")
    t_goff = nc.dram_tensor("g_off", [128, NCH], I32, kind="ExternalInput")
    t_dstrel = nc.dram_tensor("dstrel", [128, NCH], F32, kind="ExternalInput")
    t_attrT = nc.dram_tensor("attrT", [ED + 1, EC], BF16, kind="ExternalInput")
    t_glocal = nc.dram_tensor("glocal", [128, NW], F32, kind="ExternalInput")
    t_prows = nc.dram_tensor("pool_rows", [128, 2], I32, kind="ExternalInput")
    t_iota = nc.dram_tensor("iota", [128, 128], F32, kind="ExternalInput")
    t_iota2 = nc.dram_tensor("iota2", [128, GLW], F32, kind="ExternalInput")
    t_ident = nc.dram_tensor("ident", [128, 128], BF16, kind="ExternalInput")
    t_WeT = nc.dram_tensor("WeT", [L, ED, D], BF16, kind="ExternalInput")
    t_beb = nc.dram_tensor("beb", [L, D], BF16, kind="ExternalInput")
    t_W1f = nc.dram_tensor("W1f", [L, D, D], BF16, kind="ExternalInput")
    t_W2 = nc.dram_tensor("W2", [L, D, D], BF16, kind="ExternalInput")
    t_b1f = nc.dram_tensor("b1f", [L, D], F32, kind="ExternalInput")
    t_b2 = nc.dram_tensor("b2", [L, D], F32, kind="ExternalInput")
    t_Wc1 = nc.dram_tensor("Wc1", [D, D], BF16, kind="ExternalInput")
    t_Wc2 = nc.dram_tensor("Wc2", [D, C], BF16, kind="ExternalInput")
    t_bc1 = nc.dram_tensor("bc1", [D], F32, kind="ExternalInput")
    t_bc2 = nc.dram_tensor("bc2", [C], F32, kind="ExternalInput")

    t_out = nc.dram_tensor("out", [C, G], F32, kind="ExternalOutput")

    GROWS = G + GLW

    with tile.TileContext(nc) as tc:
        with (
            tc.tile_pool(name="persist", bufs=1) as pp,
            tc.tile_pool(name="edges", bufs=3) as ep,
            tc.tile_pool(name="small", bufs=4) as sp,
            tc.tile_pool(name="nodes", bufs=1) as np1,
            tc.tile_pool(name="pse", bufs=2, space="PSUM") as pse,
            tc.tile_pool(name="psa", bufs=2, space="PSUM") as psa,
            tc.tile_pool(name="psn", bufs=2, space="PSUM") as psn,
            tc.tile_pool(name="dram", bufs=1, space="DRAM") as dp,
        ):
            # ---- persistent tiles ----
            h_pan = pp.tile([128, NLOC], BF16, tag="h")          # node-major
            nc.sync.dma_start(h_pan[:], t_xlocN[:])
            goff_t = pp.tile([128, NCH], I32, tag="goff")
            nc.sync.dma_start(goff_t[:], t_goff[:])
            dstrel_t = pp.tile([128, NCH], F32, tag="dstrel")
            nc.sync.dma_start(dstrel_t[:], t_dstrel[:])
            glocal_t = pp.tile([128, NW], F32, tag="glocal")
            nc.sync.dma_start(glocal_t[:], t_glocal[:])
            prows_t = pp.tile([128, 2], I32, tag="prows")
            nc.sync.dma_start(prows_t[:], t_prows[:])
            iota_t = pp.tile([128, 128], F32, tag="iota")
            nc.sync.dma_start(iota_t[:], t_iota[:])
            iota2_t = pp.tile([128, GLW], F32, tag="iota2")
            nc.sync.dma_start(iota2_t[:], t_iota2[:])
            ident = pp.tile([128, 128], BF16, tag="ident")
            nc.sync.dma_start(ident[:], t_ident[:])

            biases = pp.tile([128, 8], F32, tag="biases")
            for l in range(L):
                nc.sync.dma_start(biases[:, 2 * l:2 * l + 1], t_b1f[l, :, None])
                nc.sync.dma_start(biases[:, 2 * l + 1:2 * l + 2], t_b2[l, :, None])
            nc.sync.dma_start(biases[:, 6:7], t_bc1[:, None])
            nc.sync.dma_start(biases[:C, 7:8], t_bc2[:, None])

            WCOLS = 3 * L * D + D + C
            wts = pp.tile([128, WCOLS], BF16, tag="wts")
            nc.vector.memset(wts[:], 0.0)
            for l in range(L):
                nc.sync.dma_start(wts[:ED, 3 * l * D:3 * l * D + D], t_WeT[l])
                nc.sync.dma_start(wts[ED:ED + 1, 3 * l * D:3 * l * D + D],
                                  t_beb[l, None, :])
                nc.sync.dma_start(wts[:, 3 * l * D + D:3 * l * D + 2 * D], t_W1f[l])
                nc.sync.dma_start(wts[:, 3 * l * D + 2 * D:3 * l * D + 3 * D], t_W2[l])
            nc.sync.dma_start(wts[:, 3 * L * D:3 * L * D + D], t_Wc1[:])
            nc.sync.dma_start(wts[:, 3 * L * D + D:3 * L * D + D + C], t_Wc2[:])

            agg = pp.tile([128, NLOC], BF16, tag="agg")          # node-major

            ag_in = dp.tile([NLOC, D], BF16, tag="ag_in")
            ag_out = []
            for i in range(2):
                ago = dp.tile([HROWS, D], BF16, tag=f"ag_out{i}", name=f"ag_out{i}")
                ag_out.append(ago)
            pool_dram = dp.tile([GROWS, D], F32, tag="pool_dram")
            pool_red = dp.tile([GROWS, D], F32, tag="pool_red")

            def edge_layer(l, h_src_dram):
                wcol = 3 * l * D
                for w in range(NW):
                    # gather h[src] for this window's chunks
                    hg = ep.tile([128, CPW, D], BF16, tag="hg")
                    for j in range(CPW):
                        ch = w * CPW + j
                        bi = nc.gpsimd.indirect_dma_start(
                            out=hg[:, j, :], out_offset=None, in_=h_src_dram,
                            in_offset=bass.IndirectOffsetOnAxis(
                                ap=goff_t[:, ch:ch + 1], axis=0))
                        if j % 4:
                            bi.ins.queue = f"qPoolDynamic{j % 4}"
                    at = ep.tile([ED + 1, SPW], BF16, tag="attrT")
                    nc.sync.dma_start(at[:], t_attrT[:, w * SPW:(w + 1) * SPW])
                    m = ep.tile([128, CPW, D], BF16, tag="m")
                    apsum = psa.tile([128, D], F32, tag="apsum")
                    for b in range((CPW + SUB - 1) // SUB):
                        jhi = min((b + 1) * SUB, CPW)
                        nj = jhi - b * SUB
                        eps = pse.tile([128, SUB * D], F32, tag="eps")
                        for j in range(b * SUB, jhi):
                            nc.tensor.matmul(
                                eps[:, (j - b * SUB) * D:(j - b * SUB + 1) * D],
                                at[:, j * 128:(j + 1) * 128],
                                wts[:ED + 1, wcol:wcol + D],
                                start=(j == b * SUB), stop=False)
                        # accumulate gathered h into the same psum columns
                        for j in range(b * SUB, jhi):
                            nc.tensor.matmul(
                                eps[:, (j - b * SUB) * D:(j - b * SUB + 1) * D],
                                ident[:], hg[:, j, :],
                                start=False, stop=(j == jhi - 1))
                        # m = relu(e + h), psum -> sbuf
                        nc.scalar.activation(
                            m[:, b * SUB:jhi, :].rearrange("p a d -> p (a d)"),
                            eps[:, :nj * D],
                            mybir.ActivationFunctionType.Relu)
                    # scatter: agg_psum += onehot.T @ m, chunk by chunk
                    for j in range(CPW):
                        ch = w * CPW + j
                        oh = sp.tile([128, 128], BF16, tag="oh")
                        nc.vector.tensor_scalar(
                            oh[:], iota_t[:], dstrel_t[:, ch:ch + 1], None,
                            op0=mybir.AluOpType.is_equal)
                        nc.tensor.matmul(
                            apsum[:], oh[:], m[:, j, :],
                            start=(j == 0), stop=(j == CPW - 1))
                    nc.scalar.copy(agg[:, w * 128:(w + 1) * 128], apsum[:])

            def node_mlp(l):
                zN = np1.tile([128, NLOC], BF16, tag="zN")
                nc.vector.tensor_add(zN[:], h_pan[:], agg[:])
                zD = np1.tile([128, NLOC], BF16, tag="zD")
                for w in range(NW):
                    tp = psn.tile([128, 128], BF16, tag="nps", name="tp")
                    nc.tensor.transpose(tp[:], zN[:, w * 128:(w + 1) * 128], ident[:])
                    nc.vector.tensor_copy(zD[:, w * 128:(w + 1) * 128], tp[:])
                spans = [(i * 512, 512) for i in range(NLOC // 512)]
                if NLOC % 512:
                    spans.append((NLOC - NLOC % 512, NLOC % 512))
                for (o, wd) in spans:
                    ps = psn.tile([128, 512], F32, tag="nps")
                    nc.tensor.matmul(ps[:, :wd],
                                     wts[:, 3 * l * D + D:3 * l * D + 2 * D],
                                     zD[:, o:o + wd], start=True, stop=True)
                    nc.scalar.activation(zD[:, o:o + wd], ps[:, :wd],
                                         mybir.ActivationFunctionType.Relu,
                                         bias=biases[:, 2 * l:2 * l + 1])
                for (o, wd) in spans:
                    ps = psn.tile([128, 512], F32, tag="nps")
                    nc.tensor.matmul(ps[:, :wd],
                                     wts[:, 3 * l * D + 2 * D:3 * l * D + 3 * D],
                                     zD[:, o:o + wd], start=True, stop=True)
                    nc.scalar.activation(zD[:, o:o + wd], ps[:, :wd],
                                         mybir.ActivationFunctionType.Relu,
                                         bias=biases[:, 2 * l + 1:2 * l + 2])
                # back to node-major h
                for w in range(NW):
                    tp = psn.tile([128, 128], BF16, tag="nps", name="tp")
                    nc.tensor.transpose(tp[:], zD[:, w * 128:(w + 1) * 128], ident[:])
                    nc.vector.tensor_copy(h_pan[:, w * 128:(w + 1) * 128], tp[:])

            # ---------------- layers ----------------
            for l in range(L):
                h_src = t_xfull[:] if l == 0 else ag_out[(l - 1) % 2][:]
                edge_layer(l, h_src)
                node_mlp(l)
                if l < L - 1:
                    nc.sync.dma_start(
                        ag_in[:].rearrange("(b p) d -> p b d", p=128),
                        h_pan[:].rearrange("p (b d) -> p b d", d=D))
                    nc.gpsimd.collective_compute(
                        "AllGather", mybir.AluOpType.bypass,
                        ins=[ag_in.opt()], outs=[ag_out[l % 2].opt()],
                        replica_groups=[list(range(CORES))])

            # ---------------- pooling ----------------
            pps = psa.tile([128, 2, D], F32, tag="pps")
            pps0 = pps[:, 0, :]
            pps1 = pps[:, 1, :]
            for w in range(NW):
                oh0 = sp.tile([128, 128], BF16, tag="oh")
                nc.vector.tensor_scalar(
                    oh0[:], iota2_t[:, :128], glocal_t[:, w:w + 1], None,
                    op0=mybir.AluOpType.is_equal)
                nc.tensor.matmul(pps0, oh0[:], h_pan[:, w * 128:(w + 1) * 128],
                                 start=(w == 0), stop=False)
                oh1 = sp.tile([128, 128], BF16, tag="oh")
                nc.vector.tensor_scalar(
                    oh1[:], iota2_t[:, 128:], glocal_t[:, w:w + 1], None,
                    op0=mybir.AluOpType.is_equal)
                nc.tensor.matmul(pps1, oh1[:], h_pan[:, w * 128:(w + 1) * 128],
                                 start=False, stop=(w == NW - 1))
            pool_sb = np1.tile([128, 2, D], F32, tag="pool_sb")
            nc.scalar.copy(pool_sb[:, 0, :], pps0)
            nc.scalar.copy(pool_sb[:, 1, :], pps1)

            # zero the global pooled buffer, then place partials at gbase rows
            zt = np1.tile([128, (GROWS // 128) * D], F32, tag="zt")
            nc.vector.memset(zt[:], 0.0)
            nc.sync.dma_start(
                pool_dram[:].rearrange("(a p) d -> p a d", p=128),
                zt[:].rearrange("p (a d) -> p a d", d=D))
            for i in range(2):
                nc.gpsimd.indirect_dma_start(
                    out=pool_dram[:], out_offset=bass.IndirectOffsetOnAxis(
                        ap=prows_t[:, i:i + 1], axis=0),
                    in_=pool_sb[:, i, :], in_offset=None)
            nc.gpsimd.collective_compute(
                "AllReduce", mybir.AluOpType.add,
                ins=[pool_dram.opt()], outs=[pool_red.opt()],
                replica_groups=[list(range(CORES))])

            # ---------------- classifier ----------------
            prows_n = np1.tile([128, G // 128, D], F32, tag="prows_n")
            nc.sync.dma_start(
                prows_n[:], pool_red[:G, :].rearrange("(b p) d -> p b d", p=128))
            prows_bf = np1.tile([128, G // 128, D], BF16, tag="prows_bf")
            nc.vector.tensor_copy(
                prows_bf[:].rearrange("p a d -> p (a d)"),
                prows_n[:].rearrange("p a d -> p (a d)"))
            pooled_bf = np1.tile([128, G], BF16, tag="pooled_bf")   # D-major
            for b in range(G // 128):
                tp = psn.tile([128, 128], BF16, tag="nps", name="tp")
                nc.tensor.transpose(tp[:], prows_bf[:, b, :], ident[:])
                nc.vector.tensor_copy(pooled_bf[:, b * 128:(b + 1) * 128], tp[:])
            q1 = np1.tile([128, G], BF16, tag="q1")
            for o in range(0, G, 512):
                wd = min(512, G - o)
                ps = psn.tile([128, 512], F32, tag="nps")
                nc.tensor.matmul(ps[:, :wd], wts[:, 3 * L * D:3 * L * D + D],
                                 pooled_bf[:, o:o + wd], start=True, stop=True)
                nc.scalar.activation(q1[:, o:o + wd], ps[:, :wd],
                                     mybir.ActivationFunctionType.Relu,
                                     bias=biases[:, 6:7])
            outt = np1.tile([C, G], F32, tag="outt")
            for o in range(0, G, 512):
                wd = min(512, G - o)
                ps = psn.tile([128, 512], F32, tag="nps")
                nc.tensor.matmul(ps[:C, :wd], wts[:, 3 * L * D + D:3 * L * D + D + C],
                                 q1[:, o:o + wd], start=True, stop=True)
                nc.scalar.activation(outt[:, o:o + wd], ps[:C, :wd],
                                     mybir.ActivationFunctionType.Identity,
                                     bias=biases[:C, 7:8])
            nc.sync.dma_start(t_out[:], outt[:])

    nc.compile()
    return nc


_PROGRAM_CACHE = {}


def _get_program(SPW):
    if SPW not in _PROGRAM_CACHE:
        _PROGRAM_CACHE[SPW] = build_program(SPW)
    return _PROGRAM_CACHE[SPW]


def make_in_maps(prep):
    w = prep["weights"]
    a = prep["aux"]
    in_maps = []
    for c in range(CORES):
        in_maps.append({
            "x_full": prep["x_full"],
            "x_locN": np.ascontiguousarray(prep["x_locN"][c]),
            "g_off": np.ascontiguousarray(prep["g_off"][c]),
            "dstrel": np.ascontiguousarray(prep["dstrel"][c]),
            "attrT": np.ascontiguousarray(prep["attrT"][c]),
            "glocal": np.ascontiguousarray(prep["glocal"][c]),
            "pool_rows": np.ascontiguousarray(prep["pool_rows"][c]),
            "iota": a["iota"], "iota2": a["iota2"], "ident": a["ident"],
            "WeT": w["WeT"], "beb": w["beb"], "W1f": w["W1f"], "W2": w["W2"],
            "b1f": w["b1f"], "b2": w["b2"],
            "Wc1": w["Wc1"], "Wc2": w["Wc2"], "bc1": w["bc1"], "bc2": w["bc2"],
        })
    return in_maps


def postprocess(out):
    return np.ascontiguousarray(out.T.astype(np.float32))


def kernel(**inputs):
    prep = prepare(**{k: np.asarray(v) for k, v in inputs.items()})
    nc = _get_program(prep["SPW"])
    res = bass_utils.run_bass_kernel_spmd(nc, make_in_maps(prep),
                                          core_ids=list(range(CORES)))
    return postprocess(res.results[0]["out"])

